# revision 10
# baseline (speedup 1.0000x reference)
"""Single-head attention (B=8, N=2048, D=512, fp32) on 8 TRN2 NeuronCores.

Sharding: data-parallel over batch — core i computes batch element i
end-to-end (weights replicated). Per-core pipeline, all matmuls in
float32r (full-rate PE, ~1e-4 relative rounding; x/weights DMA straight
into fp32r tiles — same bits as fp32):

  phase 1+2 (pipelined with x DMA, per 128-row tile t, V skewed by one
  tile so the PE never waits on the cross-engine xT copy):
    x_t --PE transpose--> xT chunks (d on partitions)
    V[t] = xT_t^T-contract @ Wv + bv      (seq on partitions)
    after tile 4s+3: QT/KT strip s = Wq/Wk^T-contract @ xT
  phase 3, per 512-wide q strip (skewed software pipeline, PE never
  idles):
    S^T tile [k=128, q=512] = KT-chunk^T @ QT     (accum over D chunks)
    E = exp(S^T / sqrt(D))                        (ACT, fused scale)
    O[q-tile, :]  += E-slice^T @ V[kt]            (PSUM accum over kt;
                                                   natural layout, no
                                                   output transpose)
    eacc += E[kt]                                 (DVE running sum)
    sums[q-tile] = eacc-slice^T @ ones2           (4 tiny MMs per strip)
    O tiles normalized with 1/sums (per-partition scalar) and DMAed
    straight out.
"""

import numpy as np

import concourse.bass as bass
import concourse.tile as tile
from concourse import bacc, mybir
from concourse import bass_utils
from concourse.bass import ts
from concourse.masks import make_identity
from contextlib import ExitStack

B, N, D = 8, 2048, 512
P = 128
NT = N // P      # 16 seq tiles
DC = D // P      # 4 d chunks
QS = 512         # q-strip width (one PSUM bank of fp32)
NS = N // QS     # 4 strips
SOFTMAX_SCALE = 1.0 / float(np.sqrt(D))

F32 = mybir.dt.float32
F32R = mybir.dt.float32r
AF = mybir.ActivationFunctionType


def _build():
    nc = bacc.Bacc("TRN2", target_bir_lowering=False, debug=False)

    x = nc.dram_tensor("x", [N, D], F32R, kind="ExternalInput").ap()
    wq = nc.dram_tensor("wq", [D, D], F32R, kind="ExternalInput").ap()
    bq = nc.dram_tensor("bq", [D], F32, kind="ExternalInput").ap()
    wk = nc.dram_tensor("wk", [D, D], F32R, kind="ExternalInput").ap()
    bk = nc.dram_tensor("bk", [D], F32, kind="ExternalInput").ap()
    wv = nc.dram_tensor("wv", [D, D], F32R, kind="ExternalInput").ap()
    bv = nc.dram_tensor("bv", [D], F32, kind="ExternalInput").ap()
    out = nc.dram_tensor("out", [N, D], F32, kind="ExternalOutput").ap()

    with ExitStack() as ctx:
        tc = ctx.enter_context(tile.TileContext(nc))

        const = ctx.enter_context(tc.tile_pool(name="const", bufs=1))
        io512 = ctx.enter_context(tc.tile_pool(name="io512", bufs=4))
        wpool = ctx.enter_context(tc.tile_pool(name="wpool", bufs=1))
        big = ctx.enter_context(tc.tile_pool(name="big", bufs=1))
        epool = ctx.enter_context(tc.tile_pool(name="epool", bufs=3))
        accpool = ctx.enter_context(tc.tile_pool(name="accpool", bufs=2))
        opool = ctx.enter_context(tc.tile_pool(name="opool", bufs=3))
        rpool = ctx.enter_context(tc.tile_pool(name="rpool", bufs=2))

        # constants first so the identity is ready when tile 0 lands
        # (DVE copies produce the fp32r-rounded versions)
        ident_f = const.tile([P, P], F32)
        make_identity(nc, ident_f)
        ident = const.tile([P, P], F32R)
        nc.vector.tensor_copy(out=ident[:], in_=ident_f[:])
        # fp32r matmuls need even free sizes: use a 2-wide ones column
        ones_f = const.tile([P, 2], F32)
        nc.vector.memset(ones_f, 1.0)
        ones2 = const.tile([P, 2], F32R)
        nc.vector.tensor_copy(out=ones2[:], in_=ones_f[:])

        # ---- bulk DMAs: x on the SP queue (4 singles for a fast start,
        # then 3 groups of 4), weights on the ACT queue ----
        x_singles = []
        for t in range(4):
            x1 = io512.tile([P, D], F32R, tag="x1", name=f"x1_{t}")
            nc.sync.dma_start(x1[:], x[ts(t, P), :])
            x_singles.append(x1)
        x_groups = []
        for g in range(1, NT // 4):
            xg = io512.tile([P, 4, D], F32R, tag="x4", bufs=3, name=f"xg_{g}")
            nc.sync.dma_start(
                xg[:], x[ts(g, 4 * P), :].rearrange("(tt p) d -> p tt d", p=P))
            x_groups.append(xg)

        w_sb = {}
        for name, wap in (("v", wv), ("q", wq), ("k", wk)):
            wst = wpool.tile([P, DC, D], F32R, name=f"w_{name}")
            nc.scalar.dma_start(wst[:], wap.rearrange("(ko ki) d -> ki ko d", ki=P))
            w_sb[name] = wst[:]

        # biases (strided gathers) on the idle gpsimd SWDGE queue so they
        # never block the weight or x streams
        bq_sb = const.tile([P, DC], F32)
        nc.gpsimd.dma_start(bq_sb[:], bq.rearrange("(c p) -> p c", p=P))
        bk_sb = const.tile([P, DC], F32)
        nc.gpsimd.dma_start(bk_sb[:], bk.rearrange("(c p) -> p c", p=P))
        bv_rep = const.tile([P, D], F32)
        nc.gpsimd.dma_start(bv_rep[:], bv[None, :].to_broadcast((P, D)))

        # big persistent tensors
        xT = big.tile([P, DC, N], F32R)    # x^T: d on partitions
        QT = big.tile([P, DC, N], F32R)
        KT = big.tile([P, DC, N], F32R)
        V = big.tile([P, NT, D], F32R)     # natural: seq on partitions

        # ---- phase 1+2: transpose per tile, V skewed one tile behind,
        # QT/KT per 4-tile strip ----
        with tc.tile_pool(name="ps_tr", bufs=2, space="PSUM") as ps_tr, \
             tc.tile_pool(name="ps_v", bufs=2, space="PSUM") as ps_v, \
             tc.tile_pool(name="ps_proj", bufs=2, space="PSUM") as ps_proj:

            def v_proj(t):
                pv = ps_v.tile([P, D], F32, tag="pv")
                for ki in range(DC):
                    nc.tensor.matmul(
                        pv[:], xT[:, ki, ts(t, P)], w_sb["v"][:, ki, :],
                        start=(ki == 0), stop=(ki == DC - 1),
                    )
                nc.vector.tensor_add(out=V[:, t, :], in0=pv[:], in1=bv_rep[:])

            def qk_proj(s):
                for name, dstT, b_sb in (("q", QT, bq_sb), ("k", KT, bk_sb)):
                    wr = w_sb[name]
                    for co in range(DC):
                        pq = ps_proj.tile([P, QS], F32, tag="proj")
                        for ki in range(DC):
                            nc.tensor.matmul(
                                pq[:], wr[:, ki, ts(co, P)],
                                xT[:, ki, ts(s, QS)],
                                start=(ki == 0), stop=(ki == DC - 1),
                            )
                        nc.scalar.activation(
                            dstT[:, co, ts(s, QS)], pq[:], AF.Identity,
                            bias=b_sb[:, co:co + 1],
                        )

            for t in range(NT):
                if t < 4:
                    x_t = x_singles[t][:]
                else:
                    x_t = x_groups[t // 4 - 1][:, t % 4, :]
                tp = ps_tr.tile([P, D], F32R, tag="tr")
                for c in range(DC):
                    nc.tensor.matmul(
                        tp[:, ts(c, P)], x_t[:, ts(c, P)], ident[:],
                        is_transpose=True, skip_group_check=True,
                    )
                # one strided copy moves all 4 chunks; alternate engines
                dst = xT[:, :, ts(t, P)]
                if t % 2 == 0:
                    nc.vector.tensor_copy(out=dst, in_=tp[:])
                else:
                    nc.scalar.copy(dst, tp[:])
                if t >= 2:
                    v_proj(t - 2)
                if t >= 5 and t % 4 == 1:
                    qk_proj(t // 4 - 1)
            v_proj(NT - 2)
            v_proj(NT - 1)
            qk_proj(NS - 1)

        # ---- phase 3: attention, one 512-wide q strip at a time ----
        with tc.tile_pool(name="ps_st", bufs=2, space="PSUM") as ps_st, \
             tc.tile_pool(name="ps_o", bufs=4, space="PSUM") as ps_o, \
             tc.tile_pool(name="ps_sums", bufs=2, space="PSUM") as ps_sums:
            for s in range(NS):
                sums_ps = ps_sums.tile([P, 2 * DC], F32, tag="sums")
                # 4 groups share this bank; a start=True MM clears the
                # whole bank, so zero it once and accumulate (start=False)
                nc.vector.memset(sums_ps, 0.0)
                o_ps = [ps_o.tile([P, QS], F32, tag="o", name=f"o_{s}_{q}")
                        for q in range(DC)]
                eacc = accpool.tile([P, QS], F32R, tag="eacc")
                e_tiles = [None] * NT

                def o_block(kt):
                    e = e_tiles[kt]
                    for qt in range(DC):
                        nc.tensor.matmul(
                            o_ps[qt][:], e[:, ts(qt, P)], V[:, kt, :],
                            start=(kt == 0), stop=(kt == NT - 1),
                            skip_group_check=True,
                        )
                    # running row-sum of E on DVE (partition dim stays k)
                    if kt == 0:
                        nc.vector.tensor_copy(out=eacc[:], in_=e[:])
                    else:
                        nc.vector.tensor_add(out=eacc[:], in0=eacc[:], in1=e[:])

                for kt in range(NT):
                    st = ps_st.tile([P, QS], F32, tag="st")
                    for c in range(DC):
                        nc.tensor.matmul(
                            st[:], KT[:, c, ts(kt, P)], QT[:, c, ts(s, QS)],
                            start=(c == 0), stop=(c == DC - 1),
                        )
                    e = epool.tile([P, QS], F32R, tag="e")
                    nc.scalar.activation(e[:], st[:], AF.Exp, scale=SOFTMAX_SCALE)
                    e_tiles[kt] = e
                    if kt >= 1:
                        o_block(kt - 1)
                o_block(NT - 1)

                # cross-partition reduce of eacc: 4 tiny MMs vs ones2
                for qt in range(DC):
                    nc.tensor.matmul(
                        sums_ps[:, 2 * qt:2 * qt + 2], eacc[:, ts(qt, P)], ones2[:],
                        start=False, stop=True,
                        skip_group_check=True,
                    )

                r = rpool.tile([P, 2 * DC], F32, tag="r")
                nc.vector.reciprocal(r[:], sums_ps[:])
                for qt in range(DC):
                    o_sb = opool.tile([P, QS], F32, tag="o_sb")
                    if s == NS - 1 and qt % 2 == 1:
                        # last strip: split normalize across engines to
                        # shorten the tail
                        nc.scalar.activation(
                            o_sb[:], o_ps[qt][:], AF.Copy,
                            scale=r[:, 2 * qt:2 * qt + 1],
                        )
                    else:
                        nc.vector.tensor_scalar_mul(
                            out=o_sb[:], in0=o_ps[qt][:],
                            scalar1=r[:, 2 * qt:2 * qt + 1],
                        )
                    if qt % 2 == 0:
                        nc.sync.dma_start(out[ts(s * DC + qt, P), :], o_sb[:])
                    else:
                        nc.scalar.dma_start(out[ts(s * DC + qt, P), :], o_sb[:])

    nc.compile()
    return nc


_CACHE = {}


def _get_nc():
    if "nc" not in _CACHE:
        _CACHE["nc"] = _build()
    return _CACHE["nc"]


def kernel(x, Wq_w, Wq_b, Wk_w, Wk_b, Wv_w, Wv_b, _trace=False, _tmpdir=None):
    nc = _get_nc()
    x = np.ascontiguousarray(np.asarray(x, dtype=np.float32))
    args = {
        "wq": Wq_w, "bq": Wq_b,
        "wk": Wk_w, "bk": Wk_b,
        "wv": Wv_w, "bv": Wv_b,
    }
    args = {k: np.ascontiguousarray(np.asarray(v, dtype=np.float32))
            for k, v in args.items()}
    in_maps = [dict(args, x=x[i]) for i in range(B)]
    res = bass_utils.run_bass_kernel_spmd(
        nc, in_maps, core_ids=list(range(B)),
        trace=_trace, tmpdir=_tmpdir,
    )
    out = np.stack([r["out"] for r in res.results], axis=0)
    if _trace:
        kernel.last_results = res
    return out


if __name__ == "__main__":
    rng = np.random.default_rng(0)
    inputs = {
        "x": rng.standard_normal((B, N, D)).astype(np.float32),
        "Wq_w": (0.02 * rng.standard_normal((D, D))).astype(np.float32),
        "Wq_b": np.zeros(D, np.float32),
        "Wk_w": (0.02 * rng.standard_normal((D, D))).astype(np.float32),
        "Wk_b": np.zeros(D, np.float32),
        "Wv_w": (0.02 * rng.standard_normal((D, D))).astype(np.float32),
        "Wv_b": np.zeros(D, np.float32),
    }
    got = kernel(**inputs)
    print("out shape:", got.shape, got.dtype)


# revision 11
# speedup vs baseline: 1.0092x; 1.0092x over previous
"""Single-head attention (B=8, N=2048, D=512, fp32) on 8 TRN2 NeuronCores.

Sharding: data-parallel over batch — core i computes batch element i
end-to-end (weights replicated). Per-core pipeline, all matmuls in
float32r (full-rate PE, ~1e-4 relative rounding; x/weights DMA straight
into fp32r tiles — same bits as fp32):

  phase 1+2 (pipelined with x DMA, per 128-row tile t, V skewed by one
  tile so the PE never waits on the cross-engine xT copy):
    x_t --PE transpose--> xT chunks (d on partitions)
    V[t] = xT_t^T-contract @ Wv + bv      (seq on partitions)
    after tile 4s+3: QT/KT strip s = Wq/Wk^T-contract @ xT
  phase 3, per 512-wide q strip (skewed software pipeline, PE never
  idles):
    S^T tile [k=128, q=512] = KT-chunk^T @ QT     (accum over D chunks)
    E = exp(S^T / sqrt(D))                        (ACT, fused scale)
    O[q-tile, :]  += E-slice^T @ V[kt]            (PSUM accum over kt;
                                                   natural layout, no
                                                   output transpose)
    eacc += E[kt]                                 (DVE running sum)
    sums[q-tile] = eacc-slice^T @ ones2           (4 tiny MMs per strip)
    O tiles normalized with 1/sums (per-partition scalar) and DMAed
    straight out.
"""

import numpy as np

import concourse.bass as bass
import concourse.tile as tile
from concourse import bacc, mybir
from concourse import bass_utils
from concourse.bass import ts
from concourse.masks import make_identity
from contextlib import ExitStack

B, N, D = 8, 2048, 512
P = 128
NT = N // P      # 16 seq tiles
DC = D // P      # 4 d chunks
QS = 512         # q-strip width (one PSUM bank of fp32)
NS = N // QS     # 4 strips
SOFTMAX_SCALE = 1.0 / float(np.sqrt(D))

F32 = mybir.dt.float32
F32R = mybir.dt.float32r
AF = mybir.ActivationFunctionType


def _build():
    nc = bacc.Bacc("TRN2", target_bir_lowering=False, debug=False)

    x = nc.dram_tensor("x", [N, D], F32R, kind="ExternalInput").ap()
    wq = nc.dram_tensor("wq", [D, D], F32R, kind="ExternalInput").ap()
    bq = nc.dram_tensor("bq", [D], F32, kind="ExternalInput").ap()
    wk = nc.dram_tensor("wk", [D, D], F32R, kind="ExternalInput").ap()
    bk = nc.dram_tensor("bk", [D], F32, kind="ExternalInput").ap()
    wv = nc.dram_tensor("wv", [D, D], F32R, kind="ExternalInput").ap()
    bv = nc.dram_tensor("bv", [D], F32, kind="ExternalInput").ap()
    out = nc.dram_tensor("out", [N, D], F32, kind="ExternalOutput").ap()

    with ExitStack() as ctx:
        tc = ctx.enter_context(tile.TileContext(nc))

        const = ctx.enter_context(tc.tile_pool(name="const", bufs=1))
        io512 = ctx.enter_context(tc.tile_pool(name="io512", bufs=4))
        wpool = ctx.enter_context(tc.tile_pool(name="wpool", bufs=1))
        big = ctx.enter_context(tc.tile_pool(name="big", bufs=1))
        epool = ctx.enter_context(tc.tile_pool(name="epool", bufs=3))
        accpool = ctx.enter_context(tc.tile_pool(name="accpool", bufs=2))
        opool = ctx.enter_context(tc.tile_pool(name="opool", bufs=3))
        rpool = ctx.enter_context(tc.tile_pool(name="rpool", bufs=2))

        # constants first so the identity is ready when tile 0 lands
        # (DVE copies produce the fp32r-rounded versions)
        ident_f = const.tile([P, P], F32)
        make_identity(nc, ident_f)
        ident = const.tile([P, P], F32R)
        nc.vector.tensor_copy(out=ident[:], in_=ident_f[:])
        # fp32r matmuls need even free sizes: use a 2-wide ones column
        ones_f = const.tile([P, 2], F32)
        nc.vector.memset(ones_f, 1.0)
        ones2 = const.tile([P, 2], F32R)
        nc.vector.tensor_copy(out=ones2[:], in_=ones_f[:])

        # ---- bulk DMAs: x on the SP queue (4 singles for a fast start,
        # then 3 groups of 4), weights on the ACT queue ----
        x_singles = []
        for t in range(4):
            x1 = io512.tile([P, D], F32R, tag="x1", name=f"x1_{t}")
            nc.sync.dma_start(x1[:], x[ts(t, P), :])
            x_singles.append(x1)
        x_groups = []
        for g in range(1, NT // 4):
            xg = io512.tile([P, 4, D], F32R, tag="x4", bufs=3, name=f"xg_{g}")
            nc.sync.dma_start(
                xg[:], x[ts(g, 4 * P), :].rearrange("(tt p) d -> p tt d", p=P))
            x_groups.append(xg)

        w_sb = {}
        for name, wap in (("v", wv), ("q", wq), ("k", wk)):
            wst = wpool.tile([P, DC, D], F32R, name=f"w_{name}")
            nc.scalar.dma_start(wst[:], wap.rearrange("(ko ki) d -> ki ko d", ki=P))
            w_sb[name] = wst[:]

        # biases (strided gathers) on the idle gpsimd SWDGE queue so they
        # never block the weight or x streams
        bq_sb = const.tile([P, DC], F32)
        nc.gpsimd.dma_start(bq_sb[:], bq.rearrange("(c p) -> p c", p=P))
        bk_sb = const.tile([P, DC], F32)
        nc.gpsimd.dma_start(bk_sb[:], bk.rearrange("(c p) -> p c", p=P))
        bv_rep = const.tile([P, D], F32)
        nc.gpsimd.dma_start(bv_rep[:], bv[None, :].to_broadcast((P, D)))

        # big persistent tensors
        xT = big.tile([P, DC, N], F32R)    # x^T: d on partitions
        QT = big.tile([P, DC, N], F32R)
        KT = big.tile([P, DC, N], F32R)
        V = big.tile([P, NT, D], F32R)     # natural: seq on partitions

        # ---- phase 1+2: transpose per tile, V skewed one tile behind,
        # QT/KT per 4-tile strip ----
        with tc.tile_pool(name="ps_tr", bufs=2, space="PSUM") as ps_tr, \
             tc.tile_pool(name="ps_v", bufs=2, space="PSUM") as ps_v, \
             tc.tile_pool(name="ps_proj", bufs=2, space="PSUM") as ps_proj:
            # PE warm-up: dummy transposes while the first x tile is in
            # flight, so HAM un-throttles (1.2->2.4GHz) before real work
            wp = ps_tr.tile([P, P], F32R, tag="warm")
            for _ in range(16):
                nc.tensor.matmul(
                    wp[:], ident[:], ident[:],
                    is_transpose=True, skip_group_check=True,
                )
            nc.vector.tensor_copy(out=xT[:, 0, 0:P], in_=wp[:])

            def v_proj(t):
                pv = ps_v.tile([P, D], F32, tag="pv")
                for ki in range(DC):
                    nc.tensor.matmul(
                        pv[:], xT[:, ki, ts(t, P)], w_sb["v"][:, ki, :],
                        start=(ki == 0), stop=(ki == DC - 1),
                    )
                nc.vector.tensor_add(out=V[:, t, :], in0=pv[:], in1=bv_rep[:])

            def qk_proj(s):
                for name, dstT, b_sb in (("q", QT, bq_sb), ("k", KT, bk_sb)):
                    wr = w_sb[name]
                    for co in range(DC):
                        pq = ps_proj.tile([P, QS], F32, tag="proj")
                        for ki in range(DC):
                            nc.tensor.matmul(
                                pq[:], wr[:, ki, ts(co, P)],
                                xT[:, ki, ts(s, QS)],
                                start=(ki == 0), stop=(ki == DC - 1),
                            )
                        nc.scalar.activation(
                            dstT[:, co, ts(s, QS)], pq[:], AF.Identity,
                            bias=b_sb[:, co:co + 1],
                        )

            for t in range(NT):
                if t < 4:
                    x_t = x_singles[t][:]
                else:
                    x_t = x_groups[t // 4 - 1][:, t % 4, :]
                tp = ps_tr.tile([P, D], F32R, tag="tr")
                for c in range(DC):
                    nc.tensor.matmul(
                        tp[:, ts(c, P)], x_t[:, ts(c, P)], ident[:],
                        is_transpose=True, skip_group_check=True,
                    )
                # one strided copy moves all 4 chunks; alternate engines
                dst = xT[:, :, ts(t, P)]
                if t % 2 == 0:
                    nc.vector.tensor_copy(out=dst, in_=tp[:])
                else:
                    nc.scalar.copy(dst, tp[:])
                if t >= 1:
                    v_proj(t - 1)
                if t >= 4 and t % 4 == 0:
                    qk_proj(t // 4 - 1)
            v_proj(NT - 1)
            qk_proj(NS - 1)

        # ---- phase 3: attention, one 512-wide q strip at a time ----
        with tc.tile_pool(name="ps_st", bufs=2, space="PSUM") as ps_st, \
             tc.tile_pool(name="ps_o", bufs=4, space="PSUM") as ps_o, \
             tc.tile_pool(name="ps_sums", bufs=2, space="PSUM") as ps_sums:
            for s in range(NS):
                sums_ps = ps_sums.tile([P, 2 * DC], F32, tag="sums")
                # 4 groups share this bank; a start=True MM clears the
                # whole bank, so zero it once and accumulate (start=False)
                nc.vector.memset(sums_ps, 0.0)
                o_ps = [ps_o.tile([P, QS], F32, tag="o", name=f"o_{s}_{q}")
                        for q in range(DC)]
                eacc = accpool.tile([P, QS], F32R, tag="eacc")
                e_tiles = [None] * NT

                def o_block(kt):
                    e = e_tiles[kt]
                    for qt in range(DC):
                        nc.tensor.matmul(
                            o_ps[qt][:], e[:, ts(qt, P)], V[:, kt, :],
                            start=(kt == 0), stop=(kt == NT - 1),
                            skip_group_check=True,
                        )
                    # running row-sum of E on DVE (partition dim stays k)
                    if kt == 0:
                        nc.vector.tensor_copy(out=eacc[:], in_=e[:])
                    else:
                        nc.vector.tensor_add(out=eacc[:], in0=eacc[:], in1=e[:])

                for kt in range(NT):
                    st = ps_st.tile([P, QS], F32, tag="st")
                    for c in range(DC):
                        nc.tensor.matmul(
                            st[:], KT[:, c, ts(kt, P)], QT[:, c, ts(s, QS)],
                            start=(c == 0), stop=(c == DC - 1),
                        )
                    e = epool.tile([P, QS], F32R, tag="e")
                    nc.scalar.activation(e[:], st[:], AF.Exp, scale=SOFTMAX_SCALE)
                    e_tiles[kt] = e
                    if kt >= 1:
                        o_block(kt - 1)
                o_block(NT - 1)

                # cross-partition reduce of eacc: 4 tiny MMs vs ones2
                for qt in range(DC):
                    nc.tensor.matmul(
                        sums_ps[:, 2 * qt:2 * qt + 2], eacc[:, ts(qt, P)], ones2[:],
                        start=False, stop=True,
                        skip_group_check=True,
                    )

                r = rpool.tile([P, 2 * DC], F32, tag="r")
                nc.vector.reciprocal(r[:], sums_ps[:])
                for qt in range(DC):
                    o_sb = opool.tile([P, QS], F32, tag="o_sb")
                    if s == NS - 1 and qt % 2 == 1:
                        # last strip: split normalize across engines to
                        # shorten the tail
                        nc.scalar.activation(
                            o_sb[:], o_ps[qt][:], AF.Copy,
                            scale=r[:, 2 * qt:2 * qt + 1],
                        )
                    else:
                        nc.vector.tensor_scalar_mul(
                            out=o_sb[:], in0=o_ps[qt][:],
                            scalar1=r[:, 2 * qt:2 * qt + 1],
                        )
                    if qt % 2 == 0:
                        nc.sync.dma_start(out[ts(s * DC + qt, P), :], o_sb[:])
                    else:
                        nc.scalar.dma_start(out[ts(s * DC + qt, P), :], o_sb[:])

    nc.compile()
    return nc


_CACHE = {}


def _get_nc():
    if "nc" not in _CACHE:
        _CACHE["nc"] = _build()
    return _CACHE["nc"]


def kernel(x, Wq_w, Wq_b, Wk_w, Wk_b, Wv_w, Wv_b, _trace=False, _tmpdir=None):
    nc = _get_nc()
    x = np.ascontiguousarray(np.asarray(x, dtype=np.float32))
    args = {
        "wq": Wq_w, "bq": Wq_b,
        "wk": Wk_w, "bk": Wk_b,
        "wv": Wv_w, "bv": Wv_b,
    }
    args = {k: np.ascontiguousarray(np.asarray(v, dtype=np.float32))
            for k, v in args.items()}
    in_maps = [dict(args, x=x[i]) for i in range(B)]
    res = bass_utils.run_bass_kernel_spmd(
        nc, in_maps, core_ids=list(range(B)),
        trace=_trace, tmpdir=_tmpdir,
    )
    out = np.stack([r["out"] for r in res.results], axis=0)
    if _trace:
        kernel.last_results = res
    return out


if __name__ == "__main__":
    rng = np.random.default_rng(0)
    inputs = {
        "x": rng.standard_normal((B, N, D)).astype(np.float32),
        "Wq_w": (0.02 * rng.standard_normal((D, D))).astype(np.float32),
        "Wq_b": np.zeros(D, np.float32),
        "Wk_w": (0.02 * rng.standard_normal((D, D))).astype(np.float32),
        "Wk_b": np.zeros(D, np.float32),
        "Wv_w": (0.02 * rng.standard_normal((D, D))).astype(np.float32),
        "Wv_b": np.zeros(D, np.float32),
    }
    got = kernel(**inputs)
    print("out shape:", got.shape, got.dtype)
